# revision 1
# baseline (speedup 1.0000x reference)
"""HadamardNorm kernel for Trainium2 (8 NeuronCores, pure data parallel).

Computes y = LeakyReLU_{0.1}( FWHT_4096(x) / sqrt(4096) ) row-wise on
x of shape (4, 4096, 4096) fp32.

Math: FWHT_4096 = H64 (x) H64 (Kronecker).  Reshape each 4096-row to
X[i,64; j,64] (i = high 6 bits).  Y = H64 @ X @ H64, both H symmetric,
entries +-1 (exact in any dtype; accumulation in fp32 PSUM).

Per-core pipeline per supergroup of 16 rows (s in 2, g in 8):
  IN   [(s,ih,il) part, (g,jh,jl) free]   <- DMA (256B runs)
  T1   [(s,ih,jl), (g,jh,il)]             <- DVE 32x32 stream transpose
  PS1  [(s,ih,kl), (g,jh,il)]             <- PE: W1 = I4 (x) H32  (fp32r)
  T2   [(s,i),     (g,jh,kl)]             <- DVE stream transpose (PSUM->SBUF)
  PS2k [(s,i'),    (g,kl)] for kh in 0,1  <- PE: accumulate jh with
                                             W2p = I2 (x) H64, W2m = -W2p
  OUT  [(s,i'),    (g, kh*32+kl)]         <- ACT Lrelu(scale=1/64, alpha=0.1)
  y    <- DMA store (same access pattern as load)
"""

import numpy as np

import concourse.bass as bass
import concourse.mybir as mybir
import concourse.tile as tile
from concourse import bacc
from concourse.bass_utils import run_bass_kernel_spmd

N_CORES = 8
D = 4096
ROWS_TOTAL = 4 * 4096          # 16384 rows of 4096
ROWS_PER_CORE = ROWS_TOTAL // N_CORES  # 2048

F32 = mybir.dt.float32
F32R = mybir.dt.float32r

# supergroup: 16 rows (s in 2, g in 8); superblock: B supergroups per DMA
B = 4                           # supergroups per DMA superblock (64 rows, 1 MiB)
N_SGB = ROWS_PER_CORE // (16 * B)  # 32 superblocks per core


def _hadamard(n: int) -> np.ndarray:
    h = np.array([[1.0]], dtype=np.float32)
    while h.shape[0] < n:
        h = np.block([[h, h], [h, -h]])
    assert h.shape[0] == n
    return h.astype(np.float32)


def _inline_tensor_f32r(nc, data: np.ndarray, name: str):
    """inline_tensor with dtype float32r (same 4-byte f32 bits)."""
    import base64
    import io

    data = np.ascontiguousarray(data.astype(np.float32))
    mls = nc._tensor(name, list(data.shape), F32R, kind="Const", type="DRAM")
    buf = io.BytesIO()
    np.save(buf, data, allow_pickle=False)
    mls.file = f"{name}.npy"
    mls.ant_data = base64.standard_b64encode(buf.getvalue()).decode()
    return bass.DRamTensorHandle(name, list(data.shape), F32R)


def _build_nc():
    H32 = _hadamard(32)
    H64 = _hadamard(64)
    W1 = np.kron(np.eye(4, dtype=np.float32), H32)    # [128,128]
    W2P = np.kron(np.eye(2, dtype=np.float32), H64)   # [128,128]
    W2M = -W2P

    nc = bacc.Bacc("TRN2", target_bir_lowering=False, debug=False,
                   num_devices=N_CORES)

    # row = sgb*64 + u*16 + g*2 + s ; col = i*64 + j
    x = nc.dram_tensor("x", [N_SGB, B, 8, 2, 64, 64], F32,
                       kind="ExternalInput")
    y = nc.dram_tensor("y", [N_SGB, B, 8, 2, 64, 64], F32,
                       kind="ExternalOutput")

    w1_d = _inline_tensor_f32r(nc, W1, "w1c")
    w2p_d = _inline_tensor_f32r(nc, W2P, "w2pc")
    w2m_d = _inline_tensor_f32r(nc, W2M, "w2mc")

    with tile.TileContext(nc) as tc:
        with (
            tc.tile_pool(name="wpool", bufs=1) as wpool,
            tc.tile_pool(name="inp", bufs=2) as inp,
            tc.tile_pool(name="t1p", bufs=3) as t1p,
            tc.tile_pool(name="t1rp", bufs=3) as t1rp,
            tc.tile_pool(name="ps1p", bufs=2, space="PSUM") as ps1p,
            tc.tile_pool(name="t2p", bufs=3) as t2p,
            tc.tile_pool(name="t2rp", bufs=3) as t2rp,
            tc.tile_pool(name="ps2p", bufs=4, space="PSUM") as ps2p,
            tc.tile_pool(name="outp", bufs=2) as outp,
        ):
            w1 = wpool.tile([128, 128], F32R, tag="w1")
            w2p = wpool.tile([128, 128], F32R, tag="w2p")
            w2m = wpool.tile([128, 128], F32R, tag="w2m")
            nc.sync.dma_start(w1[:].bitcast(F32), w1_d[:].bitcast(F32))
            nc.sync.dma_start(w2p[:].bitcast(F32), w2p_d[:].bitcast(F32))
            nc.sync.dma_start(w2m[:].bitcast(F32), w2m_d[:].bitcast(F32))
            w1r = w1[:]
            w2pr = w2p[:]
            w2mr = w2m[:]

            for sgb in range(N_SGB):
                tin = inp.tile([128, 512 * B], F32, tag="tin")
                src = x[sgb].rearrange("u g s i j -> (s i) (u g) j")
                nc.sync.dma_start(
                    tin[:].rearrange("p (ug j) -> p ug j", ug=8 * B, j=64), src)
                tout = outp.tile([128, 512 * B], F32, tag="tout")
                for u in range(B):
                    t1 = t1p.tile([128, 512], F32, tag="t1")
                    nc.vector.transpose(t1[:], tin[:, u * 512:(u + 1) * 512])
                    t1r = t1rp.tile([128, 512], F32R, tag="t1r")
                    nc.scalar.activation(t1r[:], t1[:],
                                         mybir.ActivationFunctionType.Copy)

                    ps1 = ps1p.tile([128, 512], F32, tag="ps1")
                    nc.tensor.matmul(ps1[:], w1r, t1r[:],
                                     start=True, stop=True)

                    t2 = t2p.tile([128, 512], F32, tag="t2")
                    nc.vector.transpose(t2[:], ps1[:])
                    t2r = t2rp.tile([128, 512], F32R, tag="t2r")
                    nc.scalar.activation(t2r[:], t2[:],
                                         mybir.ActivationFunctionType.Copy)
                    # free layout of t2: (g,8)(jh,2)(kl,32); jh slices strided
                    t2v = t2r[:].rearrange("p (g jh kl) -> p jh g kl",
                                           g=8, jh=2, kl=32)
                    rhs0 = t2v[:, 0]
                    rhs1 = t2v[:, 1]

                    ov = tout[:, u * 512:(u + 1) * 512].rearrange(
                        "p (g kh kl) -> p kh g kl", g=8, kh=2, kl=32)
                    for kh in range(2):
                        ps2 = ps2p.tile([128, 256], F32, tag="ps2")
                        nc.tensor.matmul(ps2[:], w2pr, rhs0,
                                         start=True, stop=False)
                        nc.tensor.matmul(ps2[:], w2pr if kh == 0 else w2mr,
                                         rhs1, start=False, stop=True)
                        nc.scalar.activation(
                            ov[:, kh],
                            ps2[:].rearrange("p (g kl) -> p g kl", g=8),
                            mybir.ActivationFunctionType.Prelu,
                            bias=0.0, scale=1.0 / 64.0, alpha=0.1)
                dst = y[sgb].rearrange("u g s i j -> (s i) (u g) j")
                nc.sync.dma_start(
                    dst, tout[:].rearrange("p (ug j) -> p ug j", ug=8 * B, j=64))
    nc.finalize()
    return nc


_NC_CACHE = {}


def _get_nc():
    if "nc" not in _NC_CACHE:
        _NC_CACHE["nc"] = _build_nc()
    return _NC_CACHE["nc"]


def run(x: np.ndarray, trace: bool = False):
    """Returns (y, BassKernelResults)."""
    x = np.ascontiguousarray(x, dtype=np.float32)
    flat = x.reshape(-1, D)
    dev_shape = (N_SGB, B, 8, 2, 64, 64)
    shards = [
        np.ascontiguousarray(
            flat[c * ROWS_PER_CORE:(c + 1) * ROWS_PER_CORE]).reshape(dev_shape)
        for c in range(N_CORES)
    ]
    nc = _get_nc()
    res = run_bass_kernel_spmd(
        nc, [{"x": s} for s in shards], core_ids=list(range(N_CORES)),
        trace=trace)
    out = np.concatenate(
        [r["y"].reshape(ROWS_PER_CORE, D) for r in res.results], axis=0)
    return out.reshape(x.shape), res


def kernel(x: np.ndarray) -> np.ndarray:
    out, _ = run(x, trace=False)
    return out



# revision 11
# speedup vs baseline: 1.0984x; 1.0984x over previous
"""HadamardNorm kernel for Trainium2 (8 NeuronCores, pure data parallel).

Computes y = LeakyReLU_{0.1}( FWHT_4096(x) / sqrt(4096) ) row-wise on
x of shape (4, 4096, 4096) fp32.

Math: FWHT_4096 = H32 (x) H2 (x) H2 (x) H32 over index bits
(i:5 | j1:1 | j0:1 | jl:5).  Per 16 rows (s in 4, g in 4), tile
[128 part, 512 free]:

  IN   [(s,i) part, (g, j=128) free]     <- DMA, 512B contiguous runs
  MM1  [(s,a), (g, j1, j0, jl)]          <- PE: W = I4 (x) H32 contracts i
  T1   [(s,jl), (g, j1, j0, a)]          <- DVE 32x32 stream transpose
  MM2  [(s,bl), (g, j1, b0, a)]          <- PE: 4 accumulating matmuls
                                            (N=256) fold j0 with +-W while
                                            contracting jl
  T2   [(s,a), (g, j1, b0, bl)]          <- DVE stream transpose
  FOLD [(s,a), (g, b1, b0, bl)]          <- GpSimd/DVE: H2 on j1 (add/sub)
  OUT  Lrelu(scale=1/64, alpha=0.1)      <- ACT
  y    <- DMA store, 512B runs (b = b1*64+b0*32+bl contiguous)

All matmuls share the single weight pair +-(I4 (x) H32) in fp32r.
"""

import numpy as np

import concourse.bass as bass
import concourse.mybir as mybir
import concourse.tile as tile
from concourse import bacc
from concourse.bass_utils import run_bass_kernel_spmd

N_CORES = 8
D = 4096
ROWS_TOTAL = 4 * 4096              # 16384 rows of 4096
ROWS_PER_CORE = ROWS_TOTAL // N_CORES  # 2048

F32 = mybir.dt.float32
F32R = mybir.dt.float32r

B = 4                              # iters per DMA superblock (64 rows, 1 MiB)
N_SGB = ROWS_PER_CORE // (16 * B)  # 32 superblocks per core
# fraction of iters whose j1-fold runs on DVE instead of GpSimd
DVE_FOLD_EVERY = 0                 # 0 = all folds on GpSimd


def _hadamard(n: int) -> np.ndarray:
    h = np.array([[1.0]], dtype=np.float32)
    while h.shape[0] < n:
        h = np.block([[h, h], [h, -h]])
    return h.astype(np.float32)


def _inline_tensor_f32r(nc, data: np.ndarray, name: str):
    """inline_tensor with dtype float32r (same 4-byte f32 bits)."""
    import base64
    import io

    data = np.ascontiguousarray(data.astype(np.float32))
    mls = nc._tensor(name, list(data.shape), F32R, kind="Const", type="DRAM")
    buf = io.BytesIO()
    np.save(buf, data, allow_pickle=False)
    mls.file = f"{name}.npy"
    mls.ant_data = base64.standard_b64encode(buf.getvalue()).decode()
    return bass.DRamTensorHandle(name, list(data.shape), F32R)


def _build_nc():
    H32 = _hadamard(32)
    WP = np.kron(np.eye(4, dtype=np.float32), H32)    # [128,128]
    WM = -WP

    nc = bacc.Bacc("TRN2", target_bir_lowering=False, debug=False,
                   num_devices=N_CORES)

    # row = sgb*64 + u*16 + g*4 + s ; col = i*128 + j
    # x is declared f32r: same 4-byte bits; the PE truncates the mantissa
    # (tf32-style) which is well within the 2e-2 tolerance.
    x = nc.dram_tensor("x", [N_SGB, B, 4, 4, 32, 128], F32R,
                       kind="ExternalInput")
    y = nc.dram_tensor("y", [N_SGB, B, 4, 4, 32, 128], F32,
                       kind="ExternalOutput")

    wp_d = _inline_tensor_f32r(nc, WP, "wpc")
    wm_d = _inline_tensor_f32r(nc, WM, "wmc")

    with tile.TileContext(nc) as tc:
        with (
            tc.tile_pool(name="wpool", bufs=1) as wpool,
            tc.tile_pool(name="inp", bufs=2) as inp,
            tc.tile_pool(name="ps1p", bufs=2, space="PSUM") as ps1p,
            tc.tile_pool(name="t1fp", bufs=3) as t1fp,
            tc.tile_pool(name="t1p", bufs=3) as t1p,
            tc.tile_pool(name="ps2p", bufs=2, space="PSUM") as ps2p,
            tc.tile_pool(name="t2p", bufs=3) as t2p,
            tc.tile_pool(name="fp", bufs=3) as fp,
            tc.tile_pool(name="outp", bufs=2) as outp,
        ):
            wp = wpool.tile([128, 128], F32R, tag="wp")
            wm = wpool.tile([128, 128], F32R, tag="wm")
            nc.sync.dma_start(wp[:].bitcast(F32), wp_d[:].bitcast(F32))
            nc.sync.dma_start(wm[:].bitcast(F32), wm_d[:].bitcast(F32))
            wpr = wp[:]
            wmr = wm[:]

            for sgb in range(N_SGB):
                tin = inp.tile([128, 512 * B], F32R, tag="tin")
                src = x[sgb].rearrange("u g s i j -> (s i) (u g) j")
                nc.sync.dma_start(
                    tin[:].rearrange(
                        "p (ug j) -> p ug j", ug=4 * B, j=128), src)
                tout = outp.tile([128, 512 * B], F32, tag="tout")
                for u in range(B):
                    it = sgb * B + u
                    rhs1 = tin[:, u * 512:(u + 1) * 512]

                    ps1 = ps1p.tile([128, 512], F32, tag="ps1")
                    nc.tensor.matmul(ps1[:], wpr, rhs1,
                                     start=True, stop=True)

                    t1f = t1fp.tile([128, 512], F32, tag="t1f")
                    nc.vector.transpose(t1f[:], ps1[:])
                    t1 = t1p.tile([128, 512], F32R, tag="t1")
                    nc.scalar.activation(t1[:], t1f[:],
                                         mybir.ActivationFunctionType.Copy)

                    # t1 free: (g, j1, j0, a); rhs slices fix j0
                    t1v = t1[:].rearrange("p (g j1 j0 a) -> p j0 g j1 a",
                                          g=4, j1=2, j0=2, a=32)
                    # ps2 free: (g, j1, b0, a); out slices fix b0
                    ps2 = ps2p.tile([128, 512], F32, tag="ps2")
                    ps2v = ps2[:].rearrange("p (g j1 b0 a) -> p b0 g j1 a",
                                            g=4, j1=2, b0=2, a=32)
                    # b0=0: +j0=0 +j0=1 ; b0=1: +j0=0 -j0=1
                    nc.tensor.matmul(ps2v[:, 0], wpr, t1v[:, 0],
                                     start=True, stop=False)
                    nc.tensor.matmul(ps2v[:, 0], wpr, t1v[:, 1],
                                     start=False, stop=True)
                    nc.tensor.matmul(ps2v[:, 1], wpr, t1v[:, 0],
                                     start=True, stop=False)
                    nc.tensor.matmul(ps2v[:, 1], wmr, t1v[:, 1],
                                     start=False, stop=True)

                    # T2: [(s,bl),(g,j1,b0,a)] -> [(s,a),(g,j1,b0,bl)]
                    t2 = t2p.tile([128, 512], F32, tag="t2")
                    nc.vector.transpose(t2[:], ps2[:])

                    # H2 fold on j1: w[(g,b1,b0,bl)] = t2[j1=0] +- t2[j1=1]
                    w = fp.tile([128, 512], F32, tag="w")
                    t2v = t2[:].rearrange("p (g j1 c) -> p j1 g c",
                                          g=4, j1=2, c=64)
                    wv = w[:].rearrange("p (g b1 c) -> p b1 g c",
                                        g=4, b1=2, c=64)
                    eng = (nc.vector if (DVE_FOLD_EVERY and
                                         it % DVE_FOLD_EVERY == 0)
                           else nc.gpsimd)
                    eng.tensor_add(wv[:, 0], t2v[:, 0], t2v[:, 1])
                    eng.tensor_sub(wv[:, 1], t2v[:, 0], t2v[:, 1])

                    nc.scalar.activation(
                        tout[:, u * 512:(u + 1) * 512],
                        w[:],
                        mybir.ActivationFunctionType.Prelu,
                        bias=0.0, scale=1.0 / 64.0, alpha=0.1)
                dst = y[sgb].rearrange("u g s a b -> (s a) (u g) b")
                nc.sync.dma_start(
                    dst, tout[:].rearrange("p (ug b) -> p ug b",
                                           ug=4 * B, b=128))
    nc.finalize()
    return nc


_NC_CACHE = {}


def _get_nc():
    if "nc" not in _NC_CACHE:
        _NC_CACHE["nc"] = _build_nc()
    return _NC_CACHE["nc"]


def run(x: np.ndarray, trace: bool = False):
    """Returns (y, BassKernelResults)."""
    x = np.ascontiguousarray(x, dtype=np.float32)
    flat = x.reshape(-1, D)
    dev_shape = (N_SGB, B, 4, 4, 32, 128)
    shards = [
        np.ascontiguousarray(
            flat[c * ROWS_PER_CORE:(c + 1) * ROWS_PER_CORE]).reshape(dev_shape)
        for c in range(N_CORES)
    ]
    nc = _get_nc()
    res = run_bass_kernel_spmd(
        nc, [{"x": s} for s in shards], core_ids=list(range(N_CORES)),
        trace=trace)
    out = np.concatenate(
        [r["y"].reshape(ROWS_PER_CORE, D) for r in res.results], axis=0)
    return out.reshape(x.shape), res


def kernel(x: np.ndarray) -> np.ndarray:
    out, _ = run(x, trace=False)
    return out


# revision 14
# speedup vs baseline: 1.2672x; 1.1537x over previous
"""HadamardNorm kernel for Trainium2 (8 NeuronCores, pure data parallel).

Computes y = LeakyReLU_{0.1}( FWHT_4096(x) / sqrt(4096) ) row-wise on
x of shape (4, 4096, 4096) fp32.

Math: FWHT_4096 = H32 (x) H2 (x) H2 (x) H32 over index bits
(i:5 | j1:1 | j0:1 | jl:5).  Per 16 rows (s in 4, g in 4), tile
[128 part, 512 free]:

  IN   [(s,i) part, (g, j=128) free] bf16  <- DMA, 256B contiguous runs
  T0   [(s,jl), (g, j1, j0, i)]            <- DVE 32x32 stream transpose
  MMA  [(s,bl), (g, j1, b0, i)]            <- PE: 4 accumulating bf16
                                              matmuls (N=256) fold j0
                                              with +-W, contracting jl
  T1   [(s,i), (g, j1, b0, bl)]            <- DVE stream transpose (PSUM)
  FOLD [(s,i), (g, b1, b0, bl)] bf16       <- GpSimd/DVE: H2 on j1
  MMB  [(s,a), (g, b=128)]                 <- PE: W contracts i (N=512)
  OUT  Lrelu(scale=1/64, alpha=0.1) bf16   <- ACT (PSUM drain + cast)
  y    <- DMA store bf16, 256B runs

All matmuls are bf16 (H entries +-1 are exact; PSUM accumulates f32).
x is cast to bf16 host-side; y returns bf16 upcast to f32.  End-to-end
error ~3e-3 of max |y| against the 2e-2 tolerance.
"""

import numpy as np
import ml_dtypes

import concourse.bass as bass
import concourse.mybir as mybir
import concourse.tile as tile
from concourse import bacc
from concourse.bass_utils import run_bass_kernel_spmd

N_CORES = 8
D = 4096
ROWS_TOTAL = 4 * 4096              # 16384 rows of 4096
ROWS_PER_CORE = ROWS_TOTAL // N_CORES  # 2048

F32 = mybir.dt.float32
BF16 = mybir.dt.bfloat16

B = 4                              # iters per DMA superblock (64 rows)
N_SGB = ROWS_PER_CORE // (16 * B)  # 32 superblocks per core
FOLD_DVE_MOD = 4                   # every Nth iter folds on DVE (0=never)


def _hadamard(n: int) -> np.ndarray:
    h = np.array([[1.0]], dtype=np.float32)
    while h.shape[0] < n:
        h = np.block([[h, h], [h, -h]])
    return h.astype(np.float32)


def _build_nc():
    H32 = _hadamard(32)
    WP = np.kron(np.eye(4, dtype=np.float32), H32)    # [128,128]
    WM = -WP

    nc = bacc.Bacc("TRN2", target_bir_lowering=False, debug=False,
                   num_devices=N_CORES)

    # row = sgb*64 + u*16 + g*4 + s ; col = i*128 + j
    x = nc.dram_tensor("x", [N_SGB, B, 4, 4, 32, 128], BF16,
                       kind="ExternalInput")
    y = nc.dram_tensor("y", [N_SGB, B, 4, 4, 32, 128], BF16,
                       kind="ExternalOutput")

    wp_d = nc.inline_tensor(WP.astype(ml_dtypes.bfloat16), "wpc")
    wm_d = nc.inline_tensor(WM.astype(ml_dtypes.bfloat16), "wmc")

    with tile.TileContext(nc) as tc:
        with (
            tc.tile_pool(name="wpool", bufs=1) as wpool,
            tc.tile_pool(name="inp", bufs=3) as inp,
            tc.tile_pool(name="t0p", bufs=4) as t0p,
            tc.tile_pool(name="psap", bufs=3, space="PSUM") as psap,
            tc.tile_pool(name="t2p", bufs=4) as t2p,
            tc.tile_pool(name="vp", bufs=4) as vp,
            tc.tile_pool(name="psbp", bufs=3, space="PSUM") as psbp,
            tc.tile_pool(name="outp", bufs=3) as outp,
        ):
            wp = wpool.tile([128, 128], BF16, tag="wp")
            wm = wpool.tile([128, 128], BF16, tag="wm")
            nc.sync.dma_start(wp[:], wp_d[:])
            nc.sync.dma_start(wm[:], wm_d[:])
            wpr = wp[:]
            wmr = wm[:]

            for sgb in range(N_SGB):
                tin = inp.tile([128, 512 * B], BF16, tag="tin")
                src = x[sgb].rearrange("u g s i j -> (s i) (u g) j")
                nc.sync.dma_start(
                    tin[:].rearrange(
                        "p (ug j) -> p ug j", ug=4 * B, j=128), src)
                tout = outp.tile([128, 512 * B], BF16, tag="tout")
                for u in range(B):
                    it = sgb * B + u

                    # T0: [(s,i),(g,j1,j0,jl)] -> [(s,jl),(g,j1,j0,i)]
                    t0 = t0p.tile([128, 512], BF16, tag="t0")
                    nc.vector.transpose(t0[:],
                                        tin[:, u * 512:(u + 1) * 512])

                    # t0 free: (g, j1, j0, i); rhs slices fix j0
                    t0v = t0[:].rearrange("p (g j1 j0 i) -> p j0 g j1 i",
                                          g=4, j1=2, j0=2, i=32)
                    # psa free: (g, j1, b0, i); out slices fix b0
                    psa = psap.tile([128, 512], F32, tag="psa")
                    psav = psa[:].rearrange("p (g j1 b0 i) -> p b0 g j1 i",
                                            g=4, j1=2, b0=2, i=32)
                    # b0=0: +j0=0 +j0=1 ; b0=1: +j0=0 -j0=1
                    nc.tensor.matmul(psav[:, 0], wpr, t0v[:, 0],
                                     start=True, stop=False)
                    nc.tensor.matmul(psav[:, 0], wpr, t0v[:, 1],
                                     start=False, stop=True)
                    nc.tensor.matmul(psav[:, 1], wpr, t0v[:, 0],
                                     start=True, stop=False)
                    nc.tensor.matmul(psav[:, 1], wmr, t0v[:, 1],
                                     start=False, stop=True)

                    # T1: [(s,bl),(g,j1,b0,i)] -> [(s,i),(g,j1,b0,bl)]
                    t2 = t2p.tile([128, 512], F32, tag="t2")
                    nc.vector.transpose(t2[:], psa[:])

                    # H2 fold on j1 -> bf16 for MMB's ifmap
                    v = vp.tile([128, 512], BF16, tag="v")
                    t2v = t2[:].rearrange("p (g j1 c) -> p j1 g c",
                                          g=4, j1=2, c=64)
                    vv = v[:].rearrange("p (g b1 c) -> p b1 g c",
                                        g=4, b1=2, c=64)
                    eng = (nc.vector if (FOLD_DVE_MOD and
                                         it % FOLD_DVE_MOD == 0)
                           else nc.gpsimd)
                    eng.tensor_add(vv[:, 0], t2v[:, 0], t2v[:, 1])
                    eng.tensor_sub(vv[:, 1], t2v[:, 0], t2v[:, 1])

                    # MMB: contract i -> [(s,a),(g,b)]
                    psb = psbp.tile([128, 512], F32, tag="psb")
                    nc.tensor.matmul(psb[:], wpr, v[:],
                                     start=True, stop=True)

                    nc.scalar.activation(
                        tout[:, u * 512:(u + 1) * 512],
                        psb[:],
                        mybir.ActivationFunctionType.Prelu,
                        bias=0.0, scale=1.0 / 64.0, alpha=0.1)
                dst = y[sgb].rearrange("u g s a b -> (s a) (u g) b")
                nc.sync.dma_start(
                    dst, tout[:].rearrange("p (ug b) -> p ug b",
                                           ug=4 * B, b=128))
    nc.finalize()
    return nc


_NC_CACHE = {}


def _get_nc():
    if "nc" not in _NC_CACHE:
        _NC_CACHE["nc"] = _build_nc()
    return _NC_CACHE["nc"]


def run(x: np.ndarray, trace: bool = False):
    """Returns (y, BassKernelResults)."""
    x = np.ascontiguousarray(x, dtype=np.float32)
    flat = x.reshape(-1, D).astype(ml_dtypes.bfloat16)
    dev_shape = (N_SGB, B, 4, 4, 32, 128)
    shards = [
        np.ascontiguousarray(
            flat[c * ROWS_PER_CORE:(c + 1) * ROWS_PER_CORE]).reshape(dev_shape)
        for c in range(N_CORES)
    ]
    nc = _get_nc()
    res = run_bass_kernel_spmd(
        nc, [{"x": s} for s in shards], core_ids=list(range(N_CORES)),
        trace=trace)
    out = np.concatenate(
        [np.asarray(r["y"]).astype(np.float32).reshape(ROWS_PER_CORE, D)
         for r in res.results], axis=0)
    return out.reshape(x.shape), res


def kernel(x: np.ndarray) -> np.ndarray:
    out, _ = run(x, trace=False)
    return out


# revision 17
# speedup vs baseline: 1.5262x; 1.2043x over previous
"""HadamardNorm kernel for Trainium2 (8 NeuronCores, pure data parallel).

Computes y = LeakyReLU_{0.1}( FWHT_4096(x) / sqrt(4096) ) row-wise on
x of shape (4, 4096, 4096) fp32.

Math: FWHT_4096 = H32 (x) H2 (x) H2 (x) H32 over index bits
(i:5 | j1:1 | j0:1 | jl:5).  Per 16 rows (s in 4, g in 4), tile
[128 part, 512 free]:

  IN   [(s,i) part, (g, j=128) free] bf16  <- DMA, 256B contiguous runs
  T0   [(s,jl), (g, j1, j0, i)]            <- DVE 32x32 stream transpose
  MMA  [(s,bl), (g, j1, b0, i)]            <- PE: 4 accumulating bf16
                                              matmuls (N=256) fold j0
                                              with +-W, contracting jl
  T1   [(s,i), (g, j1, b0, bl)]            <- DVE stream transpose (PSUM)
  FOLD [(s,i), (g, b1, b0, bl)] bf16       <- GpSimd/DVE: H2 on j1
  MMB  [(s,a), (g, b=128)]                 <- PE: W contracts i (N=512)
  OUT  Lrelu(scale=1/64, alpha=0.1) bf16   <- ACT (PSUM drain + cast)
  y    <- DMA store bf16, 256B runs

All matmuls are bf16 (H entries +-1 are exact; PSUM accumulates f32).
x is cast to bf16 host-side; y returns bf16 upcast to f32.  End-to-end
error ~3e-3 of max |y| against the 2e-2 tolerance.
"""

import numpy as np
import ml_dtypes

import concourse.bass as bass
import concourse.mybir as mybir
import concourse.tile as tile
from concourse import bacc
from concourse.bass_utils import run_bass_kernel_spmd

N_CORES = 8
D = 4096
ROWS_TOTAL = 4 * 4096              # 16384 rows of 4096
ROWS_PER_CORE = ROWS_TOTAL // N_CORES  # 2048

F32 = mybir.dt.float32
BF16 = mybir.dt.bfloat16

B = 4                              # iters per DMA superblock (64 rows)
N_SGB = ROWS_PER_CORE // (16 * B)  # 32 superblocks per core
FOLD_DVE_MOD = 16                  # every Nth iter folds on DVE (0=never)


def _hadamard(n: int) -> np.ndarray:
    h = np.array([[1.0]], dtype=np.float32)
    while h.shape[0] < n:
        h = np.block([[h, h], [h, -h]])
    return h.astype(np.float32)


def _build_nc():
    H32 = _hadamard(32)
    WP = np.kron(np.eye(4, dtype=np.float32), H32)    # [128,128]
    WM = -WP

    nc = bacc.Bacc("TRN2", target_bir_lowering=False, debug=False,
                   num_devices=N_CORES)

    # row = sgb*64 + u*16 + g*4 + s ; col = i*128 + j
    x = nc.dram_tensor("x", [N_SGB, B, 4, 4, 32, 128], BF16,
                       kind="ExternalInput")
    y = nc.dram_tensor("y", [N_SGB, B, 4, 4, 32, 128], BF16,
                       kind="ExternalOutput")

    wp_d = nc.inline_tensor(WP.astype(ml_dtypes.bfloat16), "wpc")
    wm_d = nc.inline_tensor(WM.astype(ml_dtypes.bfloat16), "wmc")

    with tile.TileContext(nc) as tc:
        with (
            tc.tile_pool(name="wpool", bufs=1) as wpool,
            tc.tile_pool(name="inp", bufs=3) as inp,
            tc.tile_pool(name="t0p", bufs=8) as t0p,
            tc.tile_pool(name="psap", bufs=4, space="PSUM") as psap,
            tc.tile_pool(name="t2p", bufs=8) as t2p,
            tc.tile_pool(name="vp", bufs=8) as vp,
            tc.tile_pool(name="psbp", bufs=3, space="PSUM") as psbp,
            tc.tile_pool(name="outp", bufs=3) as outp,
        ):
            wp = wpool.tile([128, 128], BF16, tag="wp")
            wm = wpool.tile([128, 128], BF16, tag="wm")
            nc.sync.dma_start(wp[:], wp_d[:])
            nc.sync.dma_start(wm[:], wm_d[:])
            wpr = wp[:]
            wmr = wm[:]

            for sgb in range(N_SGB):
                tin = inp.tile([128, 512 * B], BF16, tag="tin")
                src = x[sgb].rearrange("u g s i j -> (s i) (u g) j")
                nc.sync.dma_start(
                    tin[:].rearrange(
                        "p (ug j) -> p ug j", ug=4 * B, j=128), src)
                tout = outp.tile([128, 512 * B], BF16, tag="tout")

                # stage-grouped emission: same-engine ops are contiguous
                # in the queues, so stage k of iter u overlaps stage k+1
                # of iter u-1 across engines.
                t0s = []
                for u in range(B):
                    # T0: [(s,i),(g,j1,j0,jl)] -> [(s,jl),(g,j1,j0,i)]
                    t0 = t0p.tile([128, 512], BF16, tag="t0")
                    nc.vector.transpose(t0[:],
                                        tin[:, u * 512:(u + 1) * 512])
                    t0s.append(t0)

                psas = []
                for u in range(B):
                    # t0 free: (g, j1, j0, i); rhs slices fix j0
                    t0v = t0s[u][:].rearrange(
                        "p (g j1 j0 i) -> p j0 g j1 i",
                        g=4, j1=2, j0=2, i=32)
                    # psa free: (g, j1, b0, i); out slices fix b0
                    psa = psap.tile([128, 512], F32, tag="psa")
                    psav = psa[:].rearrange("p (g j1 b0 i) -> p b0 g j1 i",
                                            g=4, j1=2, b0=2, i=32)
                    # b0=0: +j0=0 +j0=1 ; b0=1: +j0=0 -j0=1
                    nc.tensor.matmul(psav[:, 0], wpr, t0v[:, 0],
                                     start=True, stop=False)
                    nc.tensor.matmul(psav[:, 0], wpr, t0v[:, 1],
                                     start=False, stop=True)
                    nc.tensor.matmul(psav[:, 1], wpr, t0v[:, 0],
                                     start=True, stop=False)
                    nc.tensor.matmul(psav[:, 1], wmr, t0v[:, 1],
                                     start=False, stop=True)
                    psas.append(psa)

                t2s = []
                for u in range(B):
                    # T1: [(s,bl),(g,j1,b0,i)] -> [(s,i),(g,j1,b0,bl)]
                    t2 = t2p.tile([128, 512], F32, tag="t2")
                    nc.vector.transpose(t2[:], psas[u][:])
                    t2s.append(t2)

                vs = []
                for u in range(B):
                    it = sgb * B + u
                    # H2 fold on j1 -> bf16 for MMB's ifmap
                    v = vp.tile([128, 512], BF16, tag="v")
                    t2v = t2s[u][:].rearrange("p (g j1 c) -> p j1 g c",
                                              g=4, j1=2, c=64)
                    vv = v[:].rearrange("p (g b1 c) -> p b1 g c",
                                        g=4, b1=2, c=64)
                    eng = (nc.vector if (FOLD_DVE_MOD and
                                         it % FOLD_DVE_MOD == 0)
                           else nc.gpsimd)
                    eng.tensor_add(vv[:, 0], t2v[:, 0], t2v[:, 1])
                    eng.tensor_sub(vv[:, 1], t2v[:, 0], t2v[:, 1])
                    vs.append(v)

                for u in range(B):
                    # MMB: contract i -> [(s,a),(g,b)]
                    psb = psbp.tile([128, 512], F32, tag="psb")
                    nc.tensor.matmul(psb[:], wpr, vs[u][:],
                                     start=True, stop=True)
                    nc.scalar.activation(
                        tout[:, u * 512:(u + 1) * 512],
                        psb[:],
                        mybir.ActivationFunctionType.Prelu,
                        bias=0.0, scale=1.0 / 64.0, alpha=0.1)
                dst = y[sgb].rearrange("u g s a b -> (s a) (u g) b")
                nc.sync.dma_start(
                    dst, tout[:].rearrange("p (ug b) -> p ug b",
                                           ug=4 * B, b=128))
    nc.finalize()
    return nc


_NC_CACHE = {}


def _get_nc():
    if "nc" not in _NC_CACHE:
        _NC_CACHE["nc"] = _build_nc()
    return _NC_CACHE["nc"]


def run(x: np.ndarray, trace: bool = False):
    """Returns (y, BassKernelResults)."""
    x = np.ascontiguousarray(x, dtype=np.float32)
    flat = x.reshape(-1, D).astype(ml_dtypes.bfloat16)
    dev_shape = (N_SGB, B, 4, 4, 32, 128)
    shards = [
        np.ascontiguousarray(
            flat[c * ROWS_PER_CORE:(c + 1) * ROWS_PER_CORE]).reshape(dev_shape)
        for c in range(N_CORES)
    ]
    nc = _get_nc()
    res = run_bass_kernel_spmd(
        nc, [{"x": s} for s in shards], core_ids=list(range(N_CORES)),
        trace=trace)
    out = np.concatenate(
        [np.asarray(r["y"]).astype(np.float32).reshape(ROWS_PER_CORE, D)
         for r in res.results], axis=0)
    return out.reshape(x.shape), res


def kernel(x: np.ndarray) -> np.ndarray:
    out, _ = run(x, trace=False)
    return out


# revision 20
# speedup vs baseline: 1.5648x; 1.0253x over previous
"""HadamardNorm kernel for Trainium2 (8 NeuronCores, pure data parallel).

Computes y = LeakyReLU_{0.1}( FWHT_4096(x) / sqrt(4096) ) row-wise on
x of shape (4, 4096, 4096) fp32.

Math: FWHT_4096 = H32 (x) H2 (x) H2 (x) H32 over index bits
(i:5 | j1:1 | j0:1 | jl:5).  Per 16 rows (s in 4, g in 4), tile
[128 part, 512 free]:

  IN   [(s,i) part, (g, j=128) free] bf16  <- DMA, 256B contiguous runs
  T0   [(s,jl), (g, j1, j0, i)]            <- DVE 32x32 stream transpose
  MMA  [(s,bl), (g, j1, b0, i)]            <- PE: 4 accumulating bf16
                                              matmuls (N=256) fold j0
                                              with +-W, contracting jl
  T1   [(s,i), (g, j1, b0, bl)]            <- DVE stream transpose (PSUM)
  FOLD [(s,i), (g, b1, b0, bl)] bf16       <- GpSimd/DVE: H2 on j1
  MMB  [(s,a), (g, b=128)]                 <- PE: W contracts i (N=512)
  OUT  Lrelu(scale=1/64, alpha=0.1) bf16   <- ACT (PSUM drain + cast)
  y    <- DMA store bf16, 256B runs

All matmuls are bf16 (H entries +-1 are exact; PSUM accumulates f32).
x is cast to bf16 host-side; y returns bf16 upcast to f32.  End-to-end
error ~3e-3 of max |y| against the 2e-2 tolerance.
"""

import numpy as np
import ml_dtypes

import concourse.bass as bass
import concourse.mybir as mybir
import concourse.tile as tile
from concourse import bacc
from concourse.bass_utils import run_bass_kernel_spmd

N_CORES = 8
D = 4096
ROWS_TOTAL = 4 * 4096              # 16384 rows of 4096
ROWS_PER_CORE = ROWS_TOTAL // N_CORES  # 2048

F32 = mybir.dt.float32
BF16 = mybir.dt.bfloat16

B = 4                              # iters per DMA superblock (64 rows)
N_SGB = ROWS_PER_CORE // (16 * B)  # 32 superblocks per core
FOLD_DVE_MOD = 16                  # every Nth iter folds on DVE (0=never)


def _hadamard(n: int) -> np.ndarray:
    h = np.array([[1.0]], dtype=np.float32)
    while h.shape[0] < n:
        h = np.block([[h, h], [h, -h]])
    return h.astype(np.float32)


def _build_nc():
    H32 = _hadamard(32)
    WP = np.kron(np.eye(4, dtype=np.float32), H32)    # [128,128]
    WM = -WP

    nc = bacc.Bacc("TRN2", target_bir_lowering=False, debug=False,
                   num_devices=N_CORES)

    # row = sgb*64 + u*16 + g*4 + s ; col = i*128 + j
    x = nc.dram_tensor("x", [N_SGB, B, 4, 4, 32, 128], BF16,
                       kind="ExternalInput")
    y = nc.dram_tensor("y", [N_SGB, B, 4, 4, 32, 128], BF16,
                       kind="ExternalOutput")

    wp_d = nc.inline_tensor(WP.astype(ml_dtypes.bfloat16), "wpc")
    wm_d = nc.inline_tensor(WM.astype(ml_dtypes.bfloat16), "wmc")

    with tile.TileContext(nc) as tc:
        with (
            tc.tile_pool(name="wpool", bufs=1) as wpool,
            tc.tile_pool(name="inp", bufs=3) as inp,
            tc.tile_pool(name="t0p", bufs=8) as t0p,
            tc.tile_pool(name="psap", bufs=5, space="PSUM") as psap,
            tc.tile_pool(name="t2p", bufs=8) as t2p,
            tc.tile_pool(name="vp", bufs=8) as vp,
            tc.tile_pool(name="psbp", bufs=3, space="PSUM") as psbp,
            tc.tile_pool(name="outp", bufs=8) as outp,
        ):
            wp = wpool.tile([128, 128], BF16, tag="wp")
            wm = wpool.tile([128, 128], BF16, tag="wm")
            nc.sync.dma_start(wp[:], wp_d[:])
            nc.sync.dma_start(wm[:], wm_d[:])
            wpr = wp[:]
            wmr = wm[:]

            for sgb in range(N_SGB):
                tin = inp.tile([128, 512 * B], BF16, tag="tin")
                src = x[sgb].rearrange("u g s i j -> (s i) (u g) j")
                nc.sync.dma_start(
                    tin[:].rearrange(
                        "p (ug j) -> p ug j", ug=4 * B, j=128), src)
                # stage-grouped emission: same-engine ops are contiguous
                # in the queues, so stage k of iter u overlaps stage k+1
                # of iter u-1 across engines.
                t0s = []
                for u in range(B):
                    # T0: [(s,i),(g,j1,j0,jl)] -> [(s,jl), j0-major free]
                    # t0 physical free layout (j0, g, j1, i) so MMA rhs
                    # slices are contiguous.
                    t0 = t0p.tile([128, 512], BF16, tag="t0")
                    nc.vector.transpose(
                        t0[:].rearrange("p (j0 g j1 i) -> p g j1 j0 i",
                                        j0=2, g=4, j1=2, i=32),
                        tin[:, u * 512:(u + 1) * 512].rearrange(
                            "p (g j1 j0 jl) -> p g j1 j0 jl",
                            g=4, j1=2, j0=2, jl=32))
                    t0s.append(t0)

                psas = []
                for u in range(B):
                    t0v = t0s[u][:].rearrange("p (j0 c) -> p j0 c",
                                              j0=2, c=256)
                    # psa physical free layout (b0, g, j1, i): contiguous
                    # matmul output slices
                    psa = psap.tile([128, 512], F32, tag="psa")
                    psav = psa[:].rearrange("p (b0 c) -> p b0 c",
                                            b0=2, c=256)
                    # b0=0: +j0=0 +j0=1 ; b0=1: +j0=0 -j0=1
                    nc.tensor.matmul(psav[:, 0], wpr, t0v[:, 0],
                                     start=True, stop=False)
                    nc.tensor.matmul(psav[:, 0], wpr, t0v[:, 1],
                                     start=False, stop=True)
                    nc.tensor.matmul(psav[:, 1], wpr, t0v[:, 0],
                                     start=True, stop=False)
                    nc.tensor.matmul(psav[:, 1], wmr, t0v[:, 1],
                                     start=False, stop=True)
                    psas.append(psa)

                t2s = []
                for u in range(B):
                    # T1: [(s,bl),(b0,g,j1,i)] -> [(s,i), t2 (g,j1,b0,bl)]
                    t2 = t2p.tile([128, 512], F32, tag="t2")
                    nc.vector.transpose(
                        t2[:].rearrange("p (g j1 b0 bl) -> p b0 g j1 bl",
                                        g=4, j1=2, b0=2, bl=32),
                        psas[u][:].rearrange("p (b0 g j1 i) -> p b0 g j1 i",
                                             b0=2, g=4, j1=2, i=32))
                    t2s.append(t2)

                vs = []
                for u in range(B):
                    it = sgb * B + u
                    # H2 fold on j1 -> bf16 for MMB's ifmap
                    v = vp.tile([128, 512], BF16, tag="v")
                    t2v = t2s[u][:].rearrange("p (g j1 c) -> p j1 g c",
                                              g=4, j1=2, c=64)
                    vv = v[:].rearrange("p (g b1 c) -> p b1 g c",
                                        g=4, b1=2, c=64)
                    eng = (nc.vector if (FOLD_DVE_MOD and
                                         it % FOLD_DVE_MOD == 0)
                           else nc.gpsimd)
                    eng.tensor_add(vv[:, 0], t2v[:, 0], t2v[:, 1])
                    eng.tensor_sub(vv[:, 1], t2v[:, 0], t2v[:, 1])
                    vs.append(v)

                for u in range(B):
                    # MMB: contract i -> [(s,a),(g,b)]
                    psb = psbp.tile([128, 512], F32, tag="psb")
                    nc.tensor.matmul(psb[:], wpr, vs[u][:],
                                     start=True, stop=True)
                    tout = outp.tile([128, 512], BF16, tag="tout")
                    nc.scalar.activation(
                        tout[:],
                        psb[:],
                        mybir.ActivationFunctionType.Prelu,
                        bias=0.0, scale=1.0 / 64.0, alpha=0.1)
                    dst = y[sgb, u].rearrange("g s a b -> (s a) g b")
                    nc.sync.dma_start(
                        dst, tout[:].rearrange("p (g b) -> p g b",
                                               g=4, b=128))
    nc.finalize()
    return nc


_NC_CACHE = {}


def _get_nc():
    if "nc" not in _NC_CACHE:
        _NC_CACHE["nc"] = _build_nc()
    return _NC_CACHE["nc"]


def run(x: np.ndarray, trace: bool = False):
    """Returns (y, BassKernelResults)."""
    x = np.ascontiguousarray(x, dtype=np.float32)
    flat = x.reshape(-1, D).astype(ml_dtypes.bfloat16)
    dev_shape = (N_SGB, B, 4, 4, 32, 128)
    shards = [
        np.ascontiguousarray(
            flat[c * ROWS_PER_CORE:(c + 1) * ROWS_PER_CORE]).reshape(dev_shape)
        for c in range(N_CORES)
    ]
    nc = _get_nc()
    res = run_bass_kernel_spmd(
        nc, [{"x": s} for s in shards], core_ids=list(range(N_CORES)),
        trace=trace)
    out = np.concatenate(
        [np.asarray(r["y"]).astype(np.float32).reshape(ROWS_PER_CORE, D)
         for r in res.results], axis=0)
    return out.reshape(x.shape), res


def kernel(x: np.ndarray) -> np.ndarray:
    out, _ = run(x, trace=False)
    return out
